# revision 12
# baseline (speedup 1.0000x reference)
"""Distributed GQA attention block for TRN2 (8 NeuronCores).

Sharding: core = b*4 + g  (b = batch 0..1, g = kv-head-pair 0..3).
Each core computes qkv for its 8 q-heads / 2 kv-heads, full attention for
those heads, and a partial c_proj ([2048,4096]); host sums the 4 partials
per batch and adds c_proj bias.

All PE-facing tensors are bf16 (fp32 PSUM accumulation). Softmax runs
without max-subtraction (scores are O(30), safe in fp32 exp), and the
additive mask is applied as exp(s+m) = exp(s)*exp(m) with exp(m)
precomputed on host, so ScalarE exps raw PSUM scores directly.
"""
import sys, os, types

sys.path.insert(0, '/opt/trn_rl_repo')

# Inject the NTFF profile hook module that this image's antenv lacks
# (needed only when tracing; harmless otherwise).
try:
    import antenv
    if "antenv.axon_hooks" not in sys.modules:
        _m = types.ModuleType("antenv.axon_hooks")
        _m._hook = None
        def _set(h, _m=_m): _m._hook = h
        def _get(_m=_m): return _m._hook
        _m.set_axon_ntff_profile_hook = _set
        _m.get_axon_ntff_profile_hook = _get
        sys.modules["antenv.axon_hooks"] = _m
        antenv.axon_hooks = _m
        try:
            from trn_agent_boot.trn_boot import _ntff_profile_via_ctypes
            _set(_ntff_profile_via_ctypes('/opt/axon/libaxon_pjrt.so'))
        except Exception:
            pass
except Exception:
    pass

import numpy as np
import ml_dtypes

import concourse.bass as bass
import concourse.tile as tile
from concourse import bacc, mybir
from concourse.bass_utils import run_bass_kernel_spmd

BF16 = mybir.dt.bfloat16
F32 = mybir.dt.float32
BNP = ml_dtypes.bfloat16

B, S, H = 2, 2048, 4096
NH, NKV, HD = 32, 8, 128
G = NH // NKV                  # 4 q heads per kv head
QH = 8                         # q heads per core
KVH = 2                        # kv heads per core
FT = QH + KVH                  # 10 qk feature tiles per core
ST = S // 128                  # 16 s tiles
KBL = H // 128                 # 32 contraction blocks
SCQ = 256                      # qkv-phase seq chunk
NCQ = S // SCQ                 # 8
QC = 512                       # attention qs chunk
NQC = S // QC                  # 4
NT = QC // 128                 # 4 qs subtiles per chunk
ST2 = ST // 2                  # paired score-tile groups (1024-wide psum)
PC = 512                       # c_proj n chunk
NPC = H // PC                  # 8
SCALE = 1.0 / float(np.sqrt(HD))
VW = HD + 1                    # v-aug row width (ones column for softmax Z)

_CACHE = {}
LAST_EXEC_NS = None
LAST_RESULTS = None


def _build_nc():
    nc = bacc.Bacc("TRN2", target_bir_lowering=False, debug=False, num_devices=8)

    xt_e = nc.declare_dram_parameter("xt", [NCQ, 128, KBL * SCQ], BF16, isOutput=False)
    wqk_e = nc.declare_dram_parameter("wqk", [FT, 128, KBL * 128], BF16, isOutput=False)
    wv_e = nc.declare_dram_parameter("wv", [128, KBL * KVH * HD], BF16, isOutput=False)
    bqk_e = nc.declare_dram_parameter("bqk", [128, FT], F32, isOutput=False)
    bv_e = nc.declare_dram_parameter("bv", [128, KVH * HD], F32, isOutput=False)
    cos_e = nc.declare_dram_parameter("cos", [128, S], BF16, isOutput=False)
    sins_e = nc.declare_dram_parameter("sins", [128, S], BF16, isOutput=False)
    em_e = nc.declare_dram_parameter("emask", [NQC, ST2, 128, 2 * QC], BF16,
                                     isOutput=False)
    wp_e = nc.declare_dram_parameter("wp", [NPC, 128, QH * PC], BF16, isOutput=False)
    id_e = nc.declare_dram_parameter("ident", [128, 128], BF16, isOutput=False)
    out_e = nc.declare_dram_parameter("out", [S, H], F32, isOutput=True)

    ADD = mybir.AluOpType.add
    MUL = mybir.AluOpType.mult
    EXP = mybir.ActivationFunctionType.Exp

    with tile.TileContext(nc) as tc:
        from contextlib import ExitStack
        with ExitStack() as ctx:
            persist = ctx.enter_context(tc.tile_pool(name="persist", bufs=1))

            qkT = [persist.tile([128, S], BF16, tag=f"qkT{i}", name=f"qkT{i}")
                   for i in range(FT)]
            outT = [persist.tile([128, S], BF16, tag=f"outT{h}", name=f"outT{h}")
                    for h in range(QH)]
            vaug = [persist.tile([128, ST * VW], BF16, tag=f"vaug{j}", name=f"vaug{j}")
                    for j in range(KVH)]
            cos_sb = persist.tile([128, S], BF16, tag="cos", name="cos")
            sins_sb = persist.tile([128, S], BF16, tag="sins", name="sins")
            wv_sb = persist.tile([128, KBL * KVH * HD], BF16, tag="wv", name="wv")
            bqk_sb = persist.tile([128, FT], F32, tag="bqk", name="bqk")
            bv_sb = persist.tile([128, KVH * HD], F32, tag="bv", name="bv")
            id_sb = persist.tile([128, 128], BF16, tag="ident", name="ident")

            nc.sync.dma_start(out=wv_sb[:], in_=wv_e.ap())
            warm = persist.tile([128, 16], F32, tag="warm", name="warm")
            nc.vector.memset(warm[:], 0.0)
            nc.scalar.activation(warm[:], warm[:],
                                 mybir.ActivationFunctionType.Exp)
            for j in range(KVH):
                nc.vector.memset(vaug[j][:], 1.0)

            # ---------------- Phase 1: qkv matmuls + bias + rope ----------------
            # Two f-tile groups: each group's weights load once (no re-reads);
            # k-heads (f=8,9) compute and rope first so attention can begin
            # while the later q-heads still rope.
            def _rope(rp, f):
                for c4 in range(S // 512):
                    sl = slice(c4 * 512, (c4 + 1) * 512)
                    rot = rp.tile([128, 512], BF16, tag="rot", name="rot")
                    nc.sync.dma_start(out=rot[0:64, :], in_=qkT[f][64:128, sl])
                    nc.sync.dma_start(out=rot[64:128, :], in_=qkT[f][0:64, sl])
                    t1 = rp.tile([128, 512], BF16, tag="t1", name="t1")
                    nc.vector.tensor_mul(t1[:], qkT[f][:, sl], cos_sb[:, sl])
                    t2 = rp.tile([128, 512], BF16, tag="t2", name="t2")
                    nc.vector.tensor_mul(t2[:], rot[:], sins_sb[:, sl])
                    nc.vector.tensor_add(qkT[f][:, sl], t1[:], t2[:])

            rp = ctx.enter_context(tc.tile_pool(name="rope", bufs=2))
            groups = [[8, 9, 0, 1, 2], [3, 4, 5, 6, 7]]
            with tc.tile_pool(name="p1", bufs=2) as p1, \
                 tc.tile_pool(name="ps1", bufs=3, space="PSUM") as ps1:
                for gi, grp in enumerate(groups):
                    # first-needed tiles hit the DMA queues first: w[grp0] + xt0
                    wqs = {}
                    wq_t = p1.tile([128, KBL * 128], BF16, tag="w0",
                                   name="w0", bufs=1)
                    xt0 = p1.tile([128, KBL * SCQ], BF16, tag="xt", name="xt")
                    for q4 in range(4):
                        wsl = slice(q4 * KBL * 32, (q4 + 1) * KBL * 32)
                        nc.sync.dma_start(out=wq_t[:, wsl],
                                          in_=wqk_e.ap()[grp[0]][:, wsl])
                        xsl = slice(q4 * KBL * SCQ // 4, (q4 + 1) * KBL * SCQ // 4)
                        nc.sync.dma_start(out=xt0[:, xsl], in_=xt_e.ap()[0][:, xsl])
                    wqs[grp[0]] = wq_t
                    for i, f in enumerate(grp[1:], start=1):
                        wq_t = p1.tile([128, KBL * 128], BF16, tag=f"w{i}",
                                       name=f"w{i}", bufs=1)
                        nc.sync.dma_start(out=wq_t[:], in_=wqk_e.ap()[f])
                        wqs[f] = wq_t
                    if gi == 0:
                        nc.sync.dma_start(out=bqk_sb[:], in_=bqk_e.ap())
                        nc.sync.dma_start(out=bv_sb[:], in_=bv_e.ap())
                        nc.sync.dma_start(out=cos_sb[:], in_=cos_e.ap())
                        nc.sync.dma_start(out=sins_sb[:], in_=sins_e.ap())
                        nc.sync.dma_start(out=id_sb[:], in_=id_e.ap())
                    for c in range(NCQ):
                        if c == 0:
                            xt_t = xt0
                        else:
                            xt_t = p1.tile([128, KBL * SCQ], BF16, tag="xt",
                                           name="xt")
                            nc.sync.dma_start(out=xt_t[:], in_=xt_e.ap()[c])
                        # qk: out[f, s] += wqk[k, f].T @ xT[k, s]
                        for f in grp:
                            psq = ps1.tile([128, SCQ], F32, tag="psq", name="psq")
                            for k in range(KBL):
                                nc.tensor.matmul(
                                    psq[:],
                                    wqs[f][:, k * 128:(k + 1) * 128],
                                    xt_t[:, k * SCQ:(k + 1) * SCQ],
                                    start=(k == 0), stop=(k == KBL - 1))
                            dst = qkT[f][:, c * SCQ:(c + 1) * SCQ]
                            if f < QH:  # fold 1/sqrt(HD) into q
                                nc.vector.tensor_scalar(
                                    dst, psq[:], bqk_sb[:, f:f + 1], SCALE, ADD, MUL)
                            else:
                                nc.vector.tensor_scalar_add(
                                    dst, psq[:], bqk_sb[:, f:f + 1])
                        if gi == 0:
                            # v: out[s, d] += xT[k, s].T @ wv[k, d]
                            for ss in range(SCQ // 128):
                                t_idx = c * (SCQ // 128) + ss
                                psv = ps1.tile([128, KVH * HD], F32, tag="psv",
                                               name="psv")
                                for k in range(KBL):
                                    nc.tensor.matmul(
                                        psv[:],
                                        xt_t[:, k * SCQ + ss * 128:
                                             k * SCQ + ss * 128 + 128],
                                        wv_sb[:, k * KVH * HD:(k + 1) * KVH * HD],
                                        start=(k == 0), stop=(k == KBL - 1))
                                for j in range(KVH):
                                    nc.vector.tensor_add(
                                        vaug[j][:, t_idx * VW: t_idx * VW + HD],
                                        psv[:, j * HD:(j + 1) * HD],
                                        bv_sb[:, j * HD:(j + 1) * HD])
                    # rope this group (in place on qkT): q' = q*cos + shuf(q)*sins
                    for f in grp:
                        _rope(rp, f)

            # ---------------- Phase 2: attention ----------------
            with tc.tile_pool(name="p2", bufs=2) as p2, \
                 tc.tile_pool(name="p2n", bufs=3) as p2n, \
                 tc.tile_pool(name="ps_sc", bufs=2, space="PSUM") as ps_sc, \
                 tc.tile_pool(name="ps_av", bufs=2, space="PSUM") as ps_av, \
                 tc.tile_pool(name="ps_tr", bufs=2, space="PSUM") as ps_tr:
                for qc in range(NQC):
                    mts = []
                    for k2 in range(ST2):
                        mt = p2.tile([128, 2 * QC], BF16, tag=f"m{k2}", name=f"m{k2}", bufs=1)
                        nc.sync.dma_start(out=mt[:], in_=em_e.ap()[qc, k2])
                        mts.append(mt)
                    for h in range(QH):
                        kv = h // G
                        p_t = p2.tile([128, ST * QC], BF16, tag="p", name="p")
                        for k2 in range(ST2):
                            psc = ps_sc.tile([128, 2 * QC], F32, tag="sc", name="sc")
                            for half in range(2):
                                kt = 2 * k2 + half
                                nc.tensor.matmul(
                                    psc[:, half * QC:(half + 1) * QC],
                                    qkT[QH + kv][:, kt * 128:(kt + 1) * 128],
                                    qkT[h][:, qc * QC:(qc + 1) * QC],
                                    start=True, stop=True)
                            pb = p_t[:, k2 * 2 * QC:(k2 + 1) * 2 * QC]
                            nc.scalar.activation(pb, psc[:], EXP)
                            nc.vector.tensor_mul(pb, pb, mts[k2][:])
                        for qs in range(NT):
                            pav = ps_av.tile([128, VW], F32, tag="av", name="av")
                            for kt in range(ST):
                                nc.tensor.matmul(
                                    pav[:],
                                    p_t[:, kt * QC + qs * 128: kt * QC + qs * 128 + 128],
                                    vaug[kv][:, kt * VW:(kt + 1) * VW],
                                    start=(kt == 0), stop=(kt == ST - 1))
                            rc = p2n.tile([128, 1], F32, tag="rc", name="rc")
                            nc.vector.reciprocal(rc[:], pav[:, HD:HD + 1])
                            onrm = p2n.tile([128, 128], BF16, tag="onrm", name="onrm")
                            nc.vector.tensor_scalar_mul(onrm[:], pav[:, 0:HD], rc[:])
                            ptr = ps_tr.tile([128, 128], BF16, tag="tr", name="tr")
                            nc.tensor.transpose(ptr[:], onrm[:], id_sb[:])
                            nc.vector.tensor_copy(
                                outT[h][:, qc * QC + qs * 128: qc * QC + qs * 128 + 128],
                                ptr[:])

            # ---------------- Phase 3: c_proj partial ----------------
            with tc.tile_pool(name="p3", bufs=2) as p3, \
                 tc.tile_pool(name="p3o", bufs=4) as p3o, \
                 tc.tile_pool(name="ps3", bufs=4, space="PSUM") as ps3:
                for ncj in range(NPC):
                    wp_t = p3.tile([128, QH * PC], BF16, tag="wp", name="wp")
                    nc.sync.dma_start(out=wp_t[:], in_=wp_e.ap()[ncj])
                    for t in range(ST):
                        pcp = ps3.tile([128, PC], F32, tag="cp", name="cp")
                        for kb in range(QH):
                            nc.tensor.matmul(
                                pcp[:],
                                outT[kb][:, t * 128:(t + 1) * 128],
                                wp_t[:, kb * PC:(kb + 1) * PC],
                                start=(kb == 0), stop=(kb == QH - 1))
                        osb = p3o.tile([128, PC], F32, tag="osb", name="osb")
                        nc.vector.tensor_copy(osb[:], pcp[:])
                        nc.sync.dma_start(
                            out=out_e.ap()[t * 128:(t + 1) * 128,
                                           ncj * PC:(ncj + 1) * PC],
                            in_=osb[:])

    nc.compile()
    return nc


def _prep_core(b, g, hidden_states, attention_mask, em_cache,
               rope_cos, rope_sin, c_attn_w, c_attn_b, c_proj_w, c_proj_b):
    x = hidden_states[b]                                   # [S, H] f32
    xt = x.T.astype(BNP)                                   # [H, S]
    # [NCQ, 128, KBL*SCQ]: xt_t[c, p, k*SCQ+j] = xT[k*128+p, c*SCQ+j]
    xt_t = np.ascontiguousarray(
        xt.reshape(KBL, 128, NCQ, SCQ).transpose(2, 1, 0, 3).reshape(
            NCQ, 128, KBL * SCQ))

    # qk weight columns for this core (f-tiles 0..7 = q heads, 8..9 = k heads)
    cols = []
    for h in range(QH):
        j = 2 * g + h // G
        qi = h % G
        c0 = 768 * j + 128 * qi
        cols.append(np.arange(c0, c0 + 128))
    for lkv in range(KVH):
        j = 2 * g + lkv
        c0 = 768 * j + G * HD
        cols.append(np.arange(c0, c0 + 128))
    cols = np.concatenate(cols)                            # [1280]
    wqk = c_attn_w[:, cols].astype(BNP)                    # [H, 1280]
    # [FT, 128, KBL*128]: wqk_t[f, p, k*128+j] = wqk[k*128+p, f*128+j]
    wqk_t = np.ascontiguousarray(
        wqk.reshape(KBL, 128, FT, 128).transpose(2, 1, 0, 3).reshape(
            FT, 128, KBL * 128))
    bqk = np.ascontiguousarray(
        c_attn_b[cols].astype(np.float32).reshape(FT, 128).T)  # [128, FT]

    vcols = np.concatenate([
        np.arange(768 * (2 * g + lkv) + G * HD + HD,
                  768 * (2 * g + lkv) + G * HD + 2 * HD)
        for lkv in range(KVH)])                            # [256]
    wv = c_attn_w[:, vcols].astype(BNP)                    # [H, 256]
    # [128, KBL*256]: wv_t[p, k*256+j] = wv[k*128+p, j]
    wv_t = np.ascontiguousarray(
        wv.reshape(KBL, 128, KVH * HD).transpose(1, 0, 2).reshape(
            128, KBL * KVH * HD))
    bv = np.ascontiguousarray(np.broadcast_to(
        c_attn_b[vcols].astype(np.float32), (128, KVH * HD)))

    cosT = np.ascontiguousarray(rope_cos.T).astype(BNP)    # [128, S]
    sinT = rope_sin.T.copy()
    sinT[0:64, :] *= -1.0
    sinsT = np.ascontiguousarray(sinT).astype(BNP)

    wp = c_proj_w[1024 * g: 1024 * (g + 1), :].astype(BNP)  # [1024, H]
    # [NPC, 128, QH*PC]: wp_t[n, p, kb*PC+j] = wp[kb*128+p, n*PC+j]
    wp_t = np.ascontiguousarray(
        wp.reshape(QH, 128, NPC, PC).transpose(2, 1, 0, 3).reshape(
            NPC, 128, QH * PC))

    ident = np.eye(128, dtype=BNP)

    return {
        "xt": xt_t, "wqk": wqk_t, "wv": wv_t, "bqk": bqk, "bv": bv,
        "cos": cosT, "sins": sinsT, "emask": em_cache[b], "wp": wp_t,
        "ident": ident,
    }


def _emask(attention_mask, b):
    # exp(maskT) tiled [NQC, ST2, 128, 2*QC]:
    # em[qc, k2, p, t*QC+j] = exp(mask[b,0, qc*QC+j, (2*k2+t)*128+p])
    maskT = attention_mask[b, 0].T                         # [S(ks), S(qs)]
    em = np.exp(maskT, dtype=np.float32)
    em_t = np.ascontiguousarray(
        em.reshape(ST2, 2, 128, NQC, QC).transpose(3, 0, 2, 1, 4).reshape(
            NQC, ST2, 128, 2 * QC)).astype(BNP)
    return em_t


def kernel(hidden_states, attention_mask, rope_cos, rope_sin,
           c_attn_w, c_attn_b, c_proj_w, c_proj_b):
    global LAST_EXEC_NS, LAST_RESULTS
    hidden_states = np.asarray(hidden_states, dtype=np.float32)
    attention_mask = np.asarray(attention_mask, dtype=np.float32)
    rope_cos = np.asarray(rope_cos, dtype=np.float32)
    rope_sin = np.asarray(rope_sin, dtype=np.float32)
    c_attn_w = np.asarray(c_attn_w, dtype=np.float32)
    c_attn_b = np.asarray(c_attn_b, dtype=np.float32)
    c_proj_w = np.asarray(c_proj_w, dtype=np.float32)
    c_proj_b = np.asarray(c_proj_b, dtype=np.float32)

    if "nc" not in _CACHE:
        _CACHE["nc"] = _build_nc()
    nc = _CACHE["nc"]

    em_cache = [_emask(attention_mask, b) for b in range(B)]
    in_maps = []
    for core in range(8):
        b, g = divmod(core, 4)
        in_maps.append(_prep_core(b, g, hidden_states, attention_mask, em_cache,
                                  rope_cos, rope_sin, c_attn_w, c_attn_b,
                                  c_proj_w, c_proj_b))

    trace = bool(int(os.environ.get("BASS_KERNEL_TRACE", "0")))
    res = run_bass_kernel_spmd(nc, in_maps, list(range(8)), trace=trace)
    LAST_EXEC_NS = res.exec_time_ns
    LAST_RESULTS = res

    out = np.zeros((B, S, H), dtype=np.float32)
    for core in range(8):
        b = core // 4
        out[b] += res.results[core]["out"]
    out += c_proj_b[None, None, :]
    return out


# revision 13
# speedup vs baseline: 1.1883x; 1.1883x over previous
"""Distributed GQA attention block for TRN2 (8 NeuronCores).

Sharding: core = b*4 + g  (b = batch 0..1, g = kv-head-pair 0..3).
Each core computes qkv for its 8 q-heads / 2 kv-heads, full attention for
those heads, and a partial c_proj ([2048,4096]); host sums the 4 partials
per batch and adds c_proj bias.

All PE-facing tensors are bf16 (fp32 PSUM accumulation). Softmax runs
without max-subtraction (scores are O(30), safe in fp32 exp), and the
additive mask is applied as exp(s+m) = exp(s)*exp(m) with exp(m)
precomputed on host, so ScalarE exps raw PSUM scores directly.
"""
import sys, os, types

sys.path.insert(0, '/opt/trn_rl_repo')

# Inject the NTFF profile hook module that this image's antenv lacks
# (needed only when tracing; harmless otherwise).
try:
    import antenv
    if "antenv.axon_hooks" not in sys.modules:
        _m = types.ModuleType("antenv.axon_hooks")
        _m._hook = None
        def _set(h, _m=_m): _m._hook = h
        def _get(_m=_m): return _m._hook
        _m.set_axon_ntff_profile_hook = _set
        _m.get_axon_ntff_profile_hook = _get
        sys.modules["antenv.axon_hooks"] = _m
        antenv.axon_hooks = _m
        try:
            from trn_agent_boot.trn_boot import _ntff_profile_via_ctypes
            _set(_ntff_profile_via_ctypes('/opt/axon/libaxon_pjrt.so'))
        except Exception:
            pass
except Exception:
    pass

import numpy as np
import ml_dtypes

import concourse.bass as bass
import concourse.tile as tile
from concourse import bacc, mybir
from concourse.bass_utils import run_bass_kernel_spmd

BF16 = mybir.dt.bfloat16
F32 = mybir.dt.float32
BNP = ml_dtypes.bfloat16

B, S, H = 2, 2048, 4096
NH, NKV, HD = 32, 8, 128
G = NH // NKV                  # 4 q heads per kv head
QH = 8                         # q heads per core
KVH = 2                        # kv heads per core
FT = QH + KVH                  # 10 qk feature tiles per core
ST = S // 128                  # 16 s tiles
KBL = H // 128                 # 32 contraction blocks
SCQ = 256                      # qkv-phase seq chunk
NCQ = S // SCQ                 # 8
QC = 512                       # attention qs chunk
NQC = S // QC                  # 4
NT = QC // 128                 # 4 qs subtiles per chunk
ST2 = ST // 2                  # paired score-tile groups (1024-wide psum)
PC = 512                       # c_proj n chunk
NPC = H // PC                  # 8
SCALE = 1.0 / float(np.sqrt(HD))
VW = HD + 1                    # v-aug row width (ones column for softmax Z)

_CACHE = {}
LAST_EXEC_NS = None
LAST_RESULTS = None


def _build_nc():
    nc = bacc.Bacc("TRN2", target_bir_lowering=False, debug=False, num_devices=8)

    xt_e = nc.declare_dram_parameter("xt", [NCQ, 128, KBL * SCQ], BF16, isOutput=False)
    wqk_e = nc.declare_dram_parameter("wqk", [FT, 128, KBL * 128], BF16, isOutput=False)
    wv_e = nc.declare_dram_parameter("wv", [128, KBL * KVH * HD], BF16, isOutput=False)
    bqk_e = nc.declare_dram_parameter("bqk", [128, FT], F32, isOutput=False)
    bv_e = nc.declare_dram_parameter("bv", [128, KVH * HD], F32, isOutput=False)
    cos_e = nc.declare_dram_parameter("cos", [128, S], BF16, isOutput=False)
    sins_e = nc.declare_dram_parameter("sins", [128, S], BF16, isOutput=False)
    em_e = nc.declare_dram_parameter("emask", [NQC, ST2, 128, 2 * QC], BF16,
                                     isOutput=False)
    wp_e = nc.declare_dram_parameter("wp", [NPC, 128, QH * PC], BF16, isOutput=False)
    id_e = nc.declare_dram_parameter("ident", [128, 128], BF16, isOutput=False)
    out_e = nc.declare_dram_parameter("out", [S, H], F32, isOutput=True)

    ADD = mybir.AluOpType.add
    MUL = mybir.AluOpType.mult
    EXP = mybir.ActivationFunctionType.Exp

    with tile.TileContext(nc) as tc:
        from contextlib import ExitStack
        with ExitStack() as ctx:
            persist = ctx.enter_context(tc.tile_pool(name="persist", bufs=1))

            qkT = [persist.tile([128, S], BF16, tag=f"qkT{i}", name=f"qkT{i}")
                   for i in range(FT)]
            outT = [persist.tile([128, S], BF16, tag=f"outT{h}", name=f"outT{h}")
                    for h in range(QH)]
            vaug = [persist.tile([128, ST * VW], BF16, tag=f"vaug{j}", name=f"vaug{j}")
                    for j in range(KVH)]
            cos_sb = persist.tile([128, S], BF16, tag="cos", name="cos")
            sins_sb = persist.tile([128, S], BF16, tag="sins", name="sins")
            wv_sb = persist.tile([128, KBL * KVH * HD], BF16, tag="wv", name="wv")
            bqk_sb = persist.tile([128, FT], F32, tag="bqk", name="bqk")
            bv_sb = persist.tile([128, KVH * HD], F32, tag="bv", name="bv")
            id_sb = persist.tile([128, 128], BF16, tag="ident", name="ident")

            nc.sync.dma_start(out=wv_sb[:], in_=wv_e.ap())
            warm = persist.tile([128, 16], F32, tag="warm", name="warm")
            nc.vector.memset(warm[:], 0.0)
            nc.scalar.activation(warm[:], warm[:],
                                 mybir.ActivationFunctionType.Exp)
            for j in range(KVH):
                nc.vector.memset(vaug[j][:], 1.0)

            # ---------------- Phase 1: qkv matmuls + bias + rope ----------------
            # Two f-tile groups: each group's weights load once (no re-reads);
            # k-heads (f=8,9) compute and rope first so attention can begin
            # while the later q-heads still rope.
            def _rope(rp, f):
                for c4 in range(S // 512):
                    sl = slice(c4 * 512, (c4 + 1) * 512)
                    rot = rp.tile([128, 512], BF16, tag="rot", name="rot")
                    nc.sync.dma_start(out=rot[0:64, :], in_=qkT[f][64:128, sl])
                    nc.sync.dma_start(out=rot[64:128, :], in_=qkT[f][0:64, sl])
                    t1 = rp.tile([128, 512], BF16, tag="t1", name="t1")
                    nc.vector.tensor_mul(t1[:], qkT[f][:, sl], cos_sb[:, sl])
                    t2 = rp.tile([128, 512], BF16, tag="t2", name="t2")
                    nc.vector.tensor_mul(t2[:], rot[:], sins_sb[:, sl])
                    nc.vector.tensor_add(qkT[f][:, sl], t1[:], t2[:])

            rp = ctx.enter_context(tc.tile_pool(name="rope", bufs=2))
            groups = [[8, 9, 0, 1, 2], [3, 4, 5, 6, 7]]
            with tc.tile_pool(name="p1", bufs=2) as p1, \
                 tc.tile_pool(name="ps1", bufs=3, space="PSUM") as ps1:
                for gi, grp in enumerate(groups):
                    # first-needed tiles hit the DMA queues first: w[grp0] + xt0
                    wqs = {}
                    wq_t = p1.tile([128, KBL * 128], BF16, tag="w0",
                                   name="w0", bufs=1)
                    nc.sync.dma_start(out=wq_t[:], in_=wqk_e.ap()[grp[0]])
                    wqs[grp[0]] = wq_t
                    xt0 = p1.tile([128, KBL * SCQ], BF16, tag="xt", name="xt")
                    nc.sync.dma_start(out=xt0[:], in_=xt_e.ap()[0])
                    for i, f in enumerate(grp[1:], start=1):
                        wq_t = p1.tile([128, KBL * 128], BF16, tag=f"w{i}",
                                       name=f"w{i}", bufs=1)
                        nc.sync.dma_start(out=wq_t[:], in_=wqk_e.ap()[f])
                        wqs[f] = wq_t
                    if gi == 0:
                        nc.sync.dma_start(out=bqk_sb[:], in_=bqk_e.ap())
                        nc.sync.dma_start(out=bv_sb[:], in_=bv_e.ap())
                        nc.sync.dma_start(out=cos_sb[:], in_=cos_e.ap())
                        nc.sync.dma_start(out=sins_sb[:], in_=sins_e.ap())
                        nc.sync.dma_start(out=id_sb[:], in_=id_e.ap())
                    for c in range(NCQ):
                        if c == 0:
                            xt_t = xt0
                        else:
                            xt_t = p1.tile([128, KBL * SCQ], BF16, tag="xt",
                                           name="xt")
                            nc.sync.dma_start(out=xt_t[:], in_=xt_e.ap()[c])
                        # qk: out[f, s] += wqk[k, f].T @ xT[k, s]
                        for f in grp:
                            psq = ps1.tile([128, SCQ], F32, tag="psq", name="psq")
                            for k in range(KBL):
                                nc.tensor.matmul(
                                    psq[:],
                                    wqs[f][:, k * 128:(k + 1) * 128],
                                    xt_t[:, k * SCQ:(k + 1) * SCQ],
                                    start=(k == 0), stop=(k == KBL - 1))
                            dst = qkT[f][:, c * SCQ:(c + 1) * SCQ]
                            if f < QH:  # fold 1/sqrt(HD) into q
                                nc.vector.tensor_scalar(
                                    dst, psq[:], bqk_sb[:, f:f + 1], SCALE, ADD, MUL)
                            else:
                                nc.vector.tensor_scalar_add(
                                    dst, psq[:], bqk_sb[:, f:f + 1])
                        if gi == 0:
                            # v: out[s, d] += xT[k, s].T @ wv[k, d]
                            for ss in range(SCQ // 128):
                                t_idx = c * (SCQ // 128) + ss
                                psv = ps1.tile([128, KVH * HD], F32, tag="psv",
                                               name="psv")
                                for k in range(KBL):
                                    nc.tensor.matmul(
                                        psv[:],
                                        xt_t[:, k * SCQ + ss * 128:
                                             k * SCQ + ss * 128 + 128],
                                        wv_sb[:, k * KVH * HD:(k + 1) * KVH * HD],
                                        start=(k == 0), stop=(k == KBL - 1))
                                for j in range(KVH):
                                    nc.vector.tensor_add(
                                        vaug[j][:, t_idx * VW: t_idx * VW + HD],
                                        psv[:, j * HD:(j + 1) * HD],
                                        bv_sb[:, j * HD:(j + 1) * HD])
                    # rope this group (in place on qkT): q' = q*cos + shuf(q)*sins
                    for f in grp:
                        _rope(rp, f)

            # ---------------- Phase 2: attention ----------------
            with tc.tile_pool(name="p2", bufs=2) as p2, \
                 tc.tile_pool(name="p2n", bufs=3) as p2n, \
                 tc.tile_pool(name="ps_sc", bufs=2, space="PSUM") as ps_sc, \
                 tc.tile_pool(name="ps_av", bufs=2, space="PSUM") as ps_av, \
                 tc.tile_pool(name="ps_tr", bufs=2, space="PSUM") as ps_tr:
                for qc in range(NQC):
                    mts = []
                    for k2 in range(ST2):
                        mt = p2.tile([128, 2 * QC], BF16, tag=f"m{k2}", name=f"m{k2}", bufs=1)
                        nc.sync.dma_start(out=mt[:], in_=em_e.ap()[qc, k2])
                        mts.append(mt)
                    for h in range(QH):
                        kv = h // G
                        p_t = p2.tile([128, ST * QC], BF16, tag="p", name="p")
                        for k2 in range(ST2):
                            psc = ps_sc.tile([128, 2 * QC], F32, tag="sc", name="sc")
                            for half in range(2):
                                kt = 2 * k2 + half
                                nc.tensor.matmul(
                                    psc[:, half * QC:(half + 1) * QC],
                                    qkT[QH + kv][:, kt * 128:(kt + 1) * 128],
                                    qkT[h][:, qc * QC:(qc + 1) * QC],
                                    start=True, stop=True)
                            pb = p_t[:, k2 * 2 * QC:(k2 + 1) * 2 * QC]
                            nc.scalar.activation(pb, psc[:], EXP)
                            nc.vector.tensor_mul(pb, pb, mts[k2][:])
                        for qs in range(NT):
                            pav = ps_av.tile([128, VW], F32, tag="av", name="av")
                            for kt in range(ST):
                                nc.tensor.matmul(
                                    pav[:],
                                    p_t[:, kt * QC + qs * 128: kt * QC + qs * 128 + 128],
                                    vaug[kv][:, kt * VW:(kt + 1) * VW],
                                    start=(kt == 0), stop=(kt == ST - 1))
                            rc = p2n.tile([128, 1], F32, tag="rc", name="rc")
                            nc.vector.reciprocal(rc[:], pav[:, HD:HD + 1])
                            onrm = p2n.tile([128, 128], BF16, tag="onrm", name="onrm")
                            nc.vector.tensor_scalar_mul(onrm[:], pav[:, 0:HD], rc[:])
                            ptr = ps_tr.tile([128, 128], BF16, tag="tr", name="tr")
                            nc.tensor.transpose(ptr[:], onrm[:], id_sb[:])
                            nc.vector.tensor_copy(
                                outT[h][:, qc * QC + qs * 128: qc * QC + qs * 128 + 128],
                                ptr[:])

            # ---------------- Phase 3: c_proj partial ----------------
            with tc.tile_pool(name="p3", bufs=2) as p3, \
                 tc.tile_pool(name="p3o", bufs=4) as p3o, \
                 tc.tile_pool(name="ps3", bufs=4, space="PSUM") as ps3:
                for ncj in range(NPC):
                    wp_t = p3.tile([128, QH * PC], BF16, tag="wp", name="wp")
                    nc.sync.dma_start(out=wp_t[:], in_=wp_e.ap()[ncj])
                    for t in range(ST):
                        pcp = ps3.tile([128, PC], F32, tag="cp", name="cp")
                        for kb in range(QH):
                            nc.tensor.matmul(
                                pcp[:],
                                outT[kb][:, t * 128:(t + 1) * 128],
                                wp_t[:, kb * PC:(kb + 1) * PC],
                                start=(kb == 0), stop=(kb == QH - 1))
                        osb = p3o.tile([128, PC], F32, tag="osb", name="osb")
                        nc.vector.tensor_copy(osb[:], pcp[:])
                        nc.sync.dma_start(
                            out=out_e.ap()[t * 128:(t + 1) * 128,
                                           ncj * PC:(ncj + 1) * PC],
                            in_=osb[:])

    nc.compile()
    return nc


def _prep_core(b, g, hidden_states, attention_mask, em_cache,
               rope_cos, rope_sin, c_attn_w, c_attn_b, c_proj_w, c_proj_b):
    x = hidden_states[b]                                   # [S, H] f32
    xt = x.T.astype(BNP)                                   # [H, S]
    # [NCQ, 128, KBL*SCQ]: xt_t[c, p, k*SCQ+j] = xT[k*128+p, c*SCQ+j]
    xt_t = np.ascontiguousarray(
        xt.reshape(KBL, 128, NCQ, SCQ).transpose(2, 1, 0, 3).reshape(
            NCQ, 128, KBL * SCQ))

    # qk weight columns for this core (f-tiles 0..7 = q heads, 8..9 = k heads)
    cols = []
    for h in range(QH):
        j = 2 * g + h // G
        qi = h % G
        c0 = 768 * j + 128 * qi
        cols.append(np.arange(c0, c0 + 128))
    for lkv in range(KVH):
        j = 2 * g + lkv
        c0 = 768 * j + G * HD
        cols.append(np.arange(c0, c0 + 128))
    cols = np.concatenate(cols)                            # [1280]
    wqk = c_attn_w[:, cols].astype(BNP)                    # [H, 1280]
    # [FT, 128, KBL*128]: wqk_t[f, p, k*128+j] = wqk[k*128+p, f*128+j]
    wqk_t = np.ascontiguousarray(
        wqk.reshape(KBL, 128, FT, 128).transpose(2, 1, 0, 3).reshape(
            FT, 128, KBL * 128))
    bqk = np.ascontiguousarray(
        c_attn_b[cols].astype(np.float32).reshape(FT, 128).T)  # [128, FT]

    vcols = np.concatenate([
        np.arange(768 * (2 * g + lkv) + G * HD + HD,
                  768 * (2 * g + lkv) + G * HD + 2 * HD)
        for lkv in range(KVH)])                            # [256]
    wv = c_attn_w[:, vcols].astype(BNP)                    # [H, 256]
    # [128, KBL*256]: wv_t[p, k*256+j] = wv[k*128+p, j]
    wv_t = np.ascontiguousarray(
        wv.reshape(KBL, 128, KVH * HD).transpose(1, 0, 2).reshape(
            128, KBL * KVH * HD))
    bv = np.ascontiguousarray(np.broadcast_to(
        c_attn_b[vcols].astype(np.float32), (128, KVH * HD)))

    cosT = np.ascontiguousarray(rope_cos.T).astype(BNP)    # [128, S]
    sinT = rope_sin.T.copy()
    sinT[0:64, :] *= -1.0
    sinsT = np.ascontiguousarray(sinT).astype(BNP)

    wp = c_proj_w[1024 * g: 1024 * (g + 1), :].astype(BNP)  # [1024, H]
    # [NPC, 128, QH*PC]: wp_t[n, p, kb*PC+j] = wp[kb*128+p, n*PC+j]
    wp_t = np.ascontiguousarray(
        wp.reshape(QH, 128, NPC, PC).transpose(2, 1, 0, 3).reshape(
            NPC, 128, QH * PC))

    ident = np.eye(128, dtype=BNP)

    return {
        "xt": xt_t, "wqk": wqk_t, "wv": wv_t, "bqk": bqk, "bv": bv,
        "cos": cosT, "sins": sinsT, "emask": em_cache[b], "wp": wp_t,
        "ident": ident,
    }


def _emask(attention_mask, b):
    # exp(maskT) tiled [NQC, ST2, 128, 2*QC]:
    # em[qc, k2, p, t*QC+j] = exp(mask[b,0, qc*QC+j, (2*k2+t)*128+p])
    maskT = attention_mask[b, 0].T                         # [S(ks), S(qs)]
    em = np.exp(maskT, dtype=np.float32)
    em_t = np.ascontiguousarray(
        em.reshape(ST2, 2, 128, NQC, QC).transpose(3, 0, 2, 1, 4).reshape(
            NQC, ST2, 128, 2 * QC)).astype(BNP)
    return em_t


def kernel(hidden_states, attention_mask, rope_cos, rope_sin,
           c_attn_w, c_attn_b, c_proj_w, c_proj_b):
    global LAST_EXEC_NS, LAST_RESULTS
    hidden_states = np.asarray(hidden_states, dtype=np.float32)
    attention_mask = np.asarray(attention_mask, dtype=np.float32)
    rope_cos = np.asarray(rope_cos, dtype=np.float32)
    rope_sin = np.asarray(rope_sin, dtype=np.float32)
    c_attn_w = np.asarray(c_attn_w, dtype=np.float32)
    c_attn_b = np.asarray(c_attn_b, dtype=np.float32)
    c_proj_w = np.asarray(c_proj_w, dtype=np.float32)
    c_proj_b = np.asarray(c_proj_b, dtype=np.float32)

    if "nc" not in _CACHE:
        _CACHE["nc"] = _build_nc()
    nc = _CACHE["nc"]

    em_cache = [_emask(attention_mask, b) for b in range(B)]
    in_maps = []
    for core in range(8):
        b, g = divmod(core, 4)
        in_maps.append(_prep_core(b, g, hidden_states, attention_mask, em_cache,
                                  rope_cos, rope_sin, c_attn_w, c_attn_b,
                                  c_proj_w, c_proj_b))

    trace = bool(int(os.environ.get("BASS_KERNEL_TRACE", "0")))
    res = run_bass_kernel_spmd(nc, in_maps, list(range(8)), trace=trace)
    LAST_EXEC_NS = res.exec_time_ns
    LAST_RESULTS = res

    out = np.zeros((B, S, H), dtype=np.float32)
    for core in range(8):
        b = core // 4
        out[b] += res.results[core]["out"]
    out += c_proj_b[None, None, :]
    return out
